# revision 9
# baseline (speedup 1.0000x reference)
"""Gaussian upsampling embedding kernel for Trainium2 (8 NeuronCores).

Data-parallel over the batch dim: 32 batches -> 4 per core.

Math (per batch b):
  c_i   = cumsum(durs)_i - durs_i/2          (gaussian centers)
  sig_i = durs_i/2 + 1e-6
  w[t,i] = 1/(sig_i*sqrt(2pi)) * exp(-((t+0.5-c_i)/sig_i)^2/2)
  out[t,:] = sum_i w[t,i]*embed[text_i] / sum_i w[t,i]          (t < total_dur)
  out[t,:] = embed[0]                                           (t >= total_dur)

Device pipeline per batch (engines overlap under Tile):
  ACT : wT[i,t] = Derivative_Erf(s_i*tval[t] + b_i)   (= 2/sqrt(pi)*exp(-u^2)),
        restricted to the t-span where any char of the chunk has |z| < 15
        (outside, w underflows to exactly 0 in f32 = reference behavior)
  DVE : Oh[i,v] = (v == text_i) * amp_i               (amp folds the pdf prefactor
                                                       and the 2/sqrt(pi))
  PE  : W2T[v,t] = Oh^T @ wT          (contract chars; per time-chunk only the
                                       char-halves whose span overlaps)
  DVE/ACT : W2T psum -> sbuf rows 0..99; row 100 = beta[t] = BIG*(t>=total_dur)
        (DMA); rows 101..127 zeroed once (persistent double-buffered tiles)
  PE  : O[t,:] = W2T^T @ embp         (embp rows: embed|embed[0]|0, cols: 384 emb
                                       + ones col (-> row-sum S) + zero pad col)
  DVE : recip[t] = 1/S (two output chunks per op via strided PSUM AP)
  DVE/ACT : out = O[:,:384]*recip  (psum->sbuf copy fused with normalize;
        whole chunks alternate between the engines to amortize op overhead)
  For t >= total_dur: O ~= BIG*embed0, S ~= BIG -> out = embed[0]
  (relative error ~1/BIG from residual gaussian tails).

float32r matmul ISA restrictions (s3d3_mm_fp32r_restrictions): stationary
operand must span all 128 PE columns (col_grp=0xf), moving/dst element counts
even -> vocab padded to 128, time padded to even, embp padded to 386 cols.
"""

import os
import numpy as np
from contextlib import ExitStack

_B, _T, _V, _D = 32, 256, 100, 384
_VP = 128           # padded vocab (fp32r needs full 128 stationary columns)
_NC = 8
_BPC = _B // _NC    # batches per core
_BIG = np.float32(1.0e6)
_EPS = np.float32(1e-6)
_MARGIN = 15.0      # |z| beyond which exp(-z^2/2) is exactly 0 in f32

# normalize chunks handled by ACT (rest on DVE); per-batch chunk index mod 10
_ACT_NORM = frozenset((1, 3, 5, 8))
# W2T psum->sbuf copy chunks handled by ACT (by matmul1 chunk index mod 4)
_ACT_CAST = frozenset((1, 3))

# Set by kernel() after each run (for the local test harness).
LAST_RESULT = None


def _grid(T2):
    """Even 256-ish boundaries for the matmul1 moving dim; last chunk merged
    so every chunk is >=256 (full-rate fp32r) and even."""
    bnds = list(range(0, T2, 256))
    if T2 - bnds[-1] < 256 and len(bnds) > 1:
        bnds.pop()
    bnds.append(T2)
    return bnds


def _build_program(Tt, spans):
    """spans[b][q] = (ci_lo, ci_hi) chunk-index range (on _grid) that char
    half q of batch-slot b contributes to; identical across cores (union)."""
    import concourse.bass as bass
    import concourse.tile as tile
    from concourse import bacc, mybir

    f32 = mybir.dt.float32
    f32r = mybir.dt.float32r
    AF = mybir.ActivationFunctionType
    ALU = mybir.AluOpType

    T2 = Tt + (Tt % 2)              # even time for fp32r moving dim
    NT = (Tt + 127) // 128          # output t-chunks of 128 rows
    NTP = NT * 128                  # padded time for matmul2 lhsT slices
    grid = _grid(T2)
    NCH = len(grid) - 1
    nfull = Tt // 128               # full 128-row output chunks

    nc = bacc.Bacc(
        "TRN2",
        target_bir_lowering=False,
        debug=False,
        num_devices=_NC,
    )

    coef = nc.dram_tensor("coef", [128, _BPC * 2 * 4], f32, kind="ExternalInput").ap()
    vrow = nc.dram_tensor("vrow", [128, _VP], f32, kind="ExternalInput").ap()
    embp = nc.dram_tensor("embp", [_VP, _D + 2], f32r, kind="ExternalInput").ap()
    beta = nc.dram_tensor("beta", [_BPC, NTP], f32r, kind="ExternalInput").ap()
    out = nc.dram_tensor("out", [_BPC, Tt, _D], f32, kind="ExternalOutput").ap()

    with tile.TileContext(nc) as tc, ExitStack() as ctx:
        const = ctx.enter_context(tc.tile_pool(name="const", bufs=1))
        wpool = ctx.enter_context(tc.tile_pool(name="wT", bufs=4))
        ohpool = ctx.enter_context(tc.tile_pool(name="oh", bufs=4))
        opool = ctx.enter_context(tc.tile_pool(name="osb", bufs=2))
        rpool = ctx.enter_context(tc.tile_pool(name="recip", bufs=6))
        psw = ctx.enter_context(tc.tile_pool(name="psw", bufs=3, space="PSUM"))
        pso = ctx.enter_context(tc.tile_pool(name="pso", bufs=2, space="PSUM"))

        # constants; small input DMAs ride the ACT HWDGE queue so they
        # don't queue behind the big output DMAs on the SP FIFO
        coef_sb = const.tile([128, _BPC * 2 * 4], f32)
        nc.sync.dma_start(coef_sb[:], coef[:])
        embp_sb = const.tile([_VP, _D + 2], f32r)
        nc.sync.dma_start(embp_sb[:], embp[:])
        vrow_sb = const.tile([128, _VP], f32)
        nc.sync.dma_start(vrow_sb[:], vrow[:])
        # tval = arange(T2)+0.5 replicated on all partitions, built on device
        tvi = const.tile([128, T2], mybir.dt.int32)
        nc.gpsimd.iota(tvi[:], [[1, T2]], channel_multiplier=0)
        tval_sb = const.tile([128, T2], f32)
        nc.vector.tensor_scalar_add(tval_sb[:], tvi[:], 0.5)

        # persistent double-buffered W2T tiles; zero fill of the constant
        # regions happens once (rows 96..127 minus the beta row, col tail)
        w2_tiles = []
        for j in range(2):
            w2 = const.tile([128, NTP], f32r, tag=f"w2_{j}")
            if NTP > T2:
                nc.gpsimd.memset(w2[:, T2:].bitcast(f32), 0.0)
            w2_tiles.append(w2)

        def cf(b, q, c):
            j = (b * 2 + q) * 4 + c
            return coef_sb[:, j : j + 1]

        for b in range(_BPC):
            # Gaussian eval restricted to contributing span
            wT = []
            for q in range(2):
                lo, hi = grid[spans[b][q][0]], grid[spans[b][q][1]]
                w = wpool.tile([128, T2], f32r, tag="wT")
                nc.scalar.activation(
                    w[:, lo:hi],
                    tval_sb[:, lo:hi],
                    AF.Derivative_Erf,
                    scale=cf(b, q, 1),
                    bias=cf(b, q, 2),
                )
                wT.append(w)

            # amplitude-scaled one-hot: Oh[q][i, v] = (vrow == text_i) * amp_i
            oh = []
            for q in range(2):
                o = ohpool.tile([128, _VP], f32r, tag="oh")
                nc.vector.tensor_scalar(
                    o[:],
                    vrow_sb[:],
                    cf(b, q, 0),
                    cf(b, q, 3),
                    ALU.is_equal,
                    ALU.mult,
                )
                oh.append(o)

            # W2T[v, t] = sum_i Oh[i, v] * wT[i, t] over contributing halves
            w2_sb = w2_tiles[b % 2]
            for ci in range(NCH):
                n0, n1 = grid[ci], grid[ci + 1]
                qs = [q for q in range(2) if spans[b][q][0] <= ci < spans[b][q][1]]
                assert qs, f"time chunk {ci} has no contributing char half"
                pw = psw.tile([128, 512], f32, tag="psw")
                for k, q in enumerate(qs):
                    nc.tensor.matmul(
                        pw[:, : n1 - n0],
                        oh[q][:],
                        wT[q][:, n0:n1],
                        start=(k == 0),
                        stop=(k == len(qs) - 1),
                    )
                if ci % 4 in _ACT_CAST:
                    nc.scalar.copy(w2_sb[:, n0:n1], pw[:, : n1 - n0])
                else:
                    nc.vector.tensor_copy(w2_sb[:, n0:n1], pw[:, : n1 - n0])
            nc.sync.dma_start(w2_sb[_V : _V + 1, :], beta[b : b + 1, :])

            # out chunks: O[t, 0:384] = unnormalized embedding, O[t, 384] = S
            # two 128-row chunks share one 2-bank psum tile -> strided recip
            out_sb = opool.tile([128, NT * _D], f32)
            for g in range((NT + 1) // 2):
                ilist = [i for i in (2 * g, 2 * g + 1) if i < NT]
                po = pso.tile([128, 1024], f32, tag="pso")
                for j, i in enumerate(ilist):
                    nc.tensor.matmul(
                        po[:, j * 512 : j * 512 + _D + 2],
                        w2_sb[:, i * 128 : (i + 1) * 128],
                        embp_sb[:],
                        start=True,
                        stop=True,
                    )
                rc = rpool.tile([128, 2], f32, tag="recip")
                ng = len(ilist)
                nc.vector.reciprocal(
                    rc[:, :ng], po[:, _D : _D + 512 * (ng - 1) + 1 : 512]
                )
                for j, i in enumerate(ilist):
                    dst = out_sb[:, i * _D : (i + 1) * _D]
                    src = po[:, j * 512 : j * 512 + _D]
                    if i % 10 in _ACT_NORM:
                        nc.scalar.activation(
                            dst, src, AF.Copy, scale=rc[:, j : j + 1]
                        )
                    else:
                        nc.vector.tensor_scalar_mul(dst, src, rc[:, j : j + 1])

            # store (two halves so the first can flush while the second
            # half of the batch is still normalizing)
            h = nfull // 2
            if h:
                nc.sync.dma_start(
                    out[b, : h * 128].rearrange("(i p) d -> p i d", p=128),
                    out_sb[:, : h * _D].rearrange("p (i d) -> p i d", d=_D),
                )
            if nfull > h:
                nc.sync.dma_start(
                    out[b, h * 128 : nfull * 128].rearrange(
                        "(i p) d -> p i d", p=128
                    ),
                    out_sb[:, h * _D : nfull * _D].rearrange(
                        "p (i d) -> p i d", d=_D
                    ),
                )
            if Tt > nfull * 128:
                rem = Tt - nfull * 128
                nc.sync.dma_start(
                    out[b, nfull * 128 :],
                    out_sb[:rem, nfull * _D : nfull * _D + _D],
                )

    nc.compile()
    return nc


def _host_prep(text, durs, embed, Tt):
    """Per-core input maps + contribution spans (chunk-index ranges)."""
    text = np.asarray(text).astype(np.float32)          # [32, 256]
    durs_f = np.asarray(durs).astype(np.float32)        # [32, 256]
    embed = np.asarray(embed, dtype=np.float32)         # [100, 384]

    T2 = Tt + (Tt % 2)
    NT = (Tt + 127) // 128
    NTP = NT * 128
    grid = _grid(T2)
    NCH = len(grid) - 1

    csum = np.cumsum(durs_f, axis=-1, dtype=np.float32)
    c = csum - durs_f / 2.0                             # centers
    sig = durs_f / 2.0 + _EPS
    sq2 = np.float32(np.sqrt(2.0))
    s_coef = (1.0 / (sig * sq2)).astype(np.float32)
    b_coef = (-c / (sig * sq2)).astype(np.float32)
    amp = (1.0 / (2.0 * sq2 * sig)).astype(np.float32)
    td = np.asarray(durs).astype(np.int64).sum(axis=-1)  # [32]

    # contribution spans per (batch, char-half): t where some |z| < _MARGIN
    lo_t = np.clip(c - _MARGIN * sig, 0, T2).reshape(_B, 2, 128).min(axis=2)
    hi_t = np.clip(c + _MARGIN * sig + 1, 0, T2).reshape(_B, 2, 128).max(axis=2)
    # union across the 8 cores (program is SPMD-shared), snap to grid chunks
    lo_s = lo_t.reshape(_NC, _BPC, 2).min(axis=0)        # [BPC, 2]
    hi_s = hi_t.reshape(_NC, _BPC, 2).max(axis=0)
    garr = np.asarray(grid)
    spans = []
    for b in range(_BPC):
        row = []
        for q in range(2):
            ci_lo = int(np.searchsorted(garr, lo_s[b, q], side="right") - 1)
            ci_hi = int(np.searchsorted(garr, hi_s[b, q], side="left"))
            ci_lo = max(0, min(ci_lo, NCH - 1))
            ci_hi = max(ci_lo + 1, min(ci_hi, NCH))
            row.append((ci_lo, ci_hi))
        spans.append(tuple(row))
    spans = tuple(spans)

    # coef layout: [128 partitions, (b, q, c)] with c = (text, s, b, amp)
    stack = np.stack([text, s_coef, b_coef, amp], axis=-1)   # [32, 256, 4]
    stack = stack.reshape(_B, 2, 128, 4)                      # [32, q, p, c]

    vrow = np.broadcast_to(
        np.arange(_VP, dtype=np.float32), (128, _VP)
    ).copy()

    # embp rows: 0..99 embed, 100 embed[0], 101..127 zero
    # cols: 0..383 embedding, 384 ones (row-sum), 385 zero pad
    embp = np.zeros((_VP, _D + 2), np.float32)
    embp[:_V, :_D] = embed
    embp[:_V, _D] = 1.0
    embp[_V, :_D] = embed[0]
    embp[_V, _D] = 1.0

    t_idx = np.arange(NTP)[None, :]
    beta_all = np.where(t_idx >= td[:, None], _BIG, np.float32(0.0)).astype(
        np.float32
    )                                                         # [32, NTP]

    in_maps = []
    for core in range(_NC):
        bs = slice(core * _BPC, (core + 1) * _BPC)
        coef_core = (
            stack[bs].transpose(2, 0, 1, 3).reshape(128, _BPC * 2 * 4).copy()
        )
        in_maps.append(
            {
                "coef": coef_core,
                "vrow": vrow,
                "embp": embp,
                "beta": beta_all[bs].copy(),
            }
        )
    return in_maps, spans


def kernel(text, durs, embed, total_time):
    global LAST_RESULT
    from concourse.bass_utils import run_bass_kernel_spmd

    Tt = int(total_time)
    in_maps, spans = _host_prep(text, durs, embed, Tt)
    nc = _build_program(Tt, spans)

    trace = bool(int(os.environ.get("GK_TRACE", "0")))
    res = run_bass_kernel_spmd(
        nc, in_maps, list(range(_NC)), trace=trace
    )
    LAST_RESULT = res
    out = np.concatenate([r["out"] for r in res.results], axis=0)
    return out.astype(np.float32)


if __name__ == "__main__":
    rng = np.random.default_rng(0)
    text = rng.integers(1, _V, size=(_B, _T), dtype=np.int64)
    durs = rng.integers(1, 9, size=(_B, _T), dtype=np.int32)
    embed = rng.normal(size=(_V, _D)).astype(np.float32)
    Tt = int(durs.sum(axis=-1).max())
    o = kernel(text, durs, embed, Tt)
    print("out", o.shape, o.dtype)


# revision 10
# speedup vs baseline: 1.0160x; 1.0160x over previous
"""Gaussian upsampling embedding kernel for Trainium2 (8 NeuronCores).

Data-parallel over the batch dim: 32 batches -> 4 per core.

Math (per batch b):
  c_i   = cumsum(durs)_i - durs_i/2          (gaussian centers)
  sig_i = durs_i/2 + 1e-6
  w[t,i] = 1/(sig_i*sqrt(2pi)) * exp(-((t+0.5-c_i)/sig_i)^2/2)
  out[t,:] = sum_i w[t,i]*embed[text_i] / sum_i w[t,i]          (t < total_dur)
  out[t,:] = embed[0]                                           (t >= total_dur)

Device pipeline per batch (engines overlap under Tile):
  ACT : wT[i,t] = Derivative_Erf(s_i*tval[t] + b_i)   (= 2/sqrt(pi)*exp(-u^2)),
        restricted to the t-span where any char of the chunk has |z| < 15
        (outside, w underflows to exactly 0 in f32 = reference behavior)
  DVE : Oh[i,v] = (v == text_i) * amp_i               (amp folds the pdf prefactor
                                                       and the 2/sqrt(pi))
  PE  : W2T[v,t] = Oh^T @ wT          (contract chars; per time-chunk only the
                                       char-halves whose span overlaps)
  DVE/ACT : W2T psum -> sbuf rows 0..99; row 100 = beta[t] = BIG*(t>=total_dur)
        (DMA); rows 101..127 zeroed once (persistent double-buffered tiles)
  PE  : O[t,:] = W2T^T @ embp         (embp rows: embed|embed[0]|0, cols: 384 emb
                                       + ones col (-> row-sum S) + zero pad col)
  DVE : recip[t] = 1/S (two output chunks per op via strided PSUM AP)
  DVE/ACT : out = O[:,:384]*recip  (psum->sbuf copy fused with normalize;
        whole chunks alternate between the engines to amortize op overhead)
  For t >= total_dur: O ~= BIG*embed0, S ~= BIG -> out = embed[0]
  (relative error ~1/BIG from residual gaussian tails).

float32r matmul ISA restrictions (s3d3_mm_fp32r_restrictions): stationary
operand must span all 128 PE columns (col_grp=0xf), moving/dst element counts
even -> vocab padded to 128, time padded to even, embp padded to 386 cols.
"""

import os
import numpy as np
from contextlib import ExitStack

_B, _T, _V, _D = 32, 256, 100, 384
_VP = 128           # padded vocab (fp32r needs full 128 stationary columns)
_NC = 8
_BPC = _B // _NC    # batches per core
_BIG = np.float32(1.0e6)
_EPS = np.float32(1e-6)
_MARGIN = 15.0      # |z| beyond which exp(-z^2/2) is exactly 0 in f32

# normalize chunks handled by ACT (rest on DVE); per-batch chunk index mod 10
_ACT_NORM = frozenset((1, 3, 5, 8))
# W2T psum->sbuf copy chunks handled by ACT (by matmul1 chunk index mod 4)
_ACT_CAST = frozenset((1, 3))

# Set by kernel() after each run (for the local test harness).
LAST_RESULT = None


def _grid(T2):
    """Even 256-ish boundaries for the matmul1 moving dim; last chunk merged
    so every chunk is >=256 (full-rate fp32r) and even."""
    bnds = list(range(0, T2, 256))
    if T2 - bnds[-1] < 256 and len(bnds) > 1:
        bnds.pop()
    bnds.append(T2)
    return bnds


def _build_program(Tt, spans):
    """spans[b][q] = (ci_lo, ci_hi) chunk-index range (on _grid) that char
    half q of batch-slot b contributes to; identical across cores (union)."""
    import concourse.bass as bass
    import concourse.tile as tile
    from concourse import bacc, mybir

    f32 = mybir.dt.float32
    f32r = mybir.dt.float32r
    AF = mybir.ActivationFunctionType
    ALU = mybir.AluOpType

    T2 = Tt + (Tt % 2)              # even time for fp32r moving dim
    NT = (Tt + 127) // 128          # output t-chunks of 128 rows
    NTP = NT * 128                  # padded time for matmul2 lhsT slices
    grid = _grid(T2)
    NCH = len(grid) - 1
    nfull = Tt // 128               # full 128-row output chunks

    nc = bacc.Bacc(
        "TRN2",
        target_bir_lowering=False,
        debug=False,
        num_devices=_NC,
    )

    coef = nc.dram_tensor("coef", [128, _BPC * 2 * 4], f32, kind="ExternalInput").ap()
    vrow = nc.dram_tensor("vrow", [128, _VP], f32, kind="ExternalInput").ap()
    embp = nc.dram_tensor("embp", [_VP, _D + 2], f32r, kind="ExternalInput").ap()
    beta = nc.dram_tensor("beta", [_BPC, NTP], f32r, kind="ExternalInput").ap()
    out = nc.dram_tensor("out", [_BPC, Tt, _D], f32, kind="ExternalOutput").ap()

    with tile.TileContext(nc) as tc, ExitStack() as ctx:
        const = ctx.enter_context(tc.tile_pool(name="const", bufs=1))
        wpool = ctx.enter_context(tc.tile_pool(name="wT", bufs=8))
        ohpool = ctx.enter_context(tc.tile_pool(name="oh", bufs=8))
        opool = ctx.enter_context(tc.tile_pool(name="osb", bufs=3))
        rpool = ctx.enter_context(tc.tile_pool(name="recip", bufs=10))
        psw = ctx.enter_context(tc.tile_pool(name="psw", bufs=3, space="PSUM"))
        pso = ctx.enter_context(tc.tile_pool(name="pso", bufs=2, space="PSUM"))

        # constants; small input DMAs ride the ACT HWDGE queue so they
        # don't queue behind the big output DMAs on the SP FIFO
        coef_sb = const.tile([128, _BPC * 2 * 4], f32)
        nc.sync.dma_start(coef_sb[:], coef[:])
        embp_sb = const.tile([_VP, _D + 2], f32r)
        nc.sync.dma_start(embp_sb[:], embp[:])
        vrow_sb = const.tile([128, _VP], f32)
        nc.sync.dma_start(vrow_sb[:], vrow[:])
        # tval = arange(T2)+0.5 replicated on all partitions, built on device
        tvi = const.tile([128, T2], mybir.dt.int32)
        nc.gpsimd.iota(tvi[:], [[1, T2]], channel_multiplier=0)
        tval_sb = const.tile([128, T2], f32)
        nc.vector.tensor_scalar_add(tval_sb[:], tvi[:], 0.5)

        # persistent double-buffered W2T tiles; zero fill of the constant
        # regions happens once (rows 96..127 minus the beta row, col tail)
        w2_tiles = []
        for j in range(_BPC):
            w2 = const.tile([128, NTP], f32r, tag=f"w2_{j}")
            if NTP > T2:
                nc.gpsimd.memset(w2[:, T2:].bitcast(f32), 0.0)
            w2_tiles.append(w2)

        def cf(b, q, c):
            j = (b * 2 + q) * 4 + c
            return coef_sb[:, j : j + 1]

        for b in range(_BPC):
            # Gaussian eval restricted to contributing span
            wT = []
            for q in range(2):
                lo, hi = grid[spans[b][q][0]], grid[spans[b][q][1]]
                w = wpool.tile([128, T2], f32r, tag="wT")
                nc.scalar.activation(
                    w[:, lo:hi],
                    tval_sb[:, lo:hi],
                    AF.Derivative_Erf,
                    scale=cf(b, q, 1),
                    bias=cf(b, q, 2),
                )
                wT.append(w)

            # amplitude-scaled one-hot: Oh[q][i, v] = (vrow == text_i) * amp_i
            oh = []
            for q in range(2):
                o = ohpool.tile([128, _VP], f32r, tag="oh")
                nc.vector.tensor_scalar(
                    o[:],
                    vrow_sb[:],
                    cf(b, q, 0),
                    cf(b, q, 3),
                    ALU.is_equal,
                    ALU.mult,
                )
                oh.append(o)

            # W2T[v, t] = sum_i Oh[i, v] * wT[i, t] over contributing halves
            w2_sb = w2_tiles[b]
            for ci in range(NCH):
                n0, n1 = grid[ci], grid[ci + 1]
                qs = [q for q in range(2) if spans[b][q][0] <= ci < spans[b][q][1]]
                assert qs, f"time chunk {ci} has no contributing char half"
                pw = psw.tile([128, 512], f32, tag="psw")
                for k, q in enumerate(qs):
                    nc.tensor.matmul(
                        pw[:, : n1 - n0],
                        oh[q][:],
                        wT[q][:, n0:n1],
                        start=(k == 0),
                        stop=(k == len(qs) - 1),
                    )
                if ci % 4 in _ACT_CAST:
                    nc.scalar.copy(w2_sb[:, n0:n1], pw[:, : n1 - n0])
                else:
                    nc.vector.tensor_copy(w2_sb[:, n0:n1], pw[:, : n1 - n0])
            nc.sync.dma_start(w2_sb[_V : _V + 1, :], beta[b : b + 1, :])

            # out chunks: O[t, 0:384] = unnormalized embedding, O[t, 384] = S
            # two 128-row chunks share one 2-bank psum tile -> strided recip
            out_sb = opool.tile([128, NT * _D], f32)
            for g in range((NT + 1) // 2):
                ilist = [i for i in (2 * g, 2 * g + 1) if i < NT]
                po = pso.tile([128, 1024], f32, tag="pso")
                for j, i in enumerate(ilist):
                    nc.tensor.matmul(
                        po[:, j * 512 : j * 512 + _D + 2],
                        w2_sb[:, i * 128 : (i + 1) * 128],
                        embp_sb[:],
                        start=True,
                        stop=True,
                    )
                rc = rpool.tile([128, 2], f32, tag="recip")
                ng = len(ilist)
                nc.vector.reciprocal(
                    rc[:, :ng], po[:, _D : _D + 512 * (ng - 1) + 1 : 512]
                )
                for j, i in enumerate(ilist):
                    dst = out_sb[:, i * _D : (i + 1) * _D]
                    src = po[:, j * 512 : j * 512 + _D]
                    if i % 10 in _ACT_NORM:
                        nc.scalar.activation(
                            dst, src, AF.Copy, scale=rc[:, j : j + 1]
                        )
                    else:
                        nc.vector.tensor_scalar_mul(dst, src, rc[:, j : j + 1])

            # store (two halves so the first can flush while the second
            # half of the batch is still normalizing)
            h = nfull // 2
            if h:
                nc.sync.dma_start(
                    out[b, : h * 128].rearrange("(i p) d -> p i d", p=128),
                    out_sb[:, : h * _D].rearrange("p (i d) -> p i d", d=_D),
                )
            if nfull > h:
                nc.sync.dma_start(
                    out[b, h * 128 : nfull * 128].rearrange(
                        "(i p) d -> p i d", p=128
                    ),
                    out_sb[:, h * _D : nfull * _D].rearrange(
                        "p (i d) -> p i d", d=_D
                    ),
                )
            if Tt > nfull * 128:
                rem = Tt - nfull * 128
                nc.sync.dma_start(
                    out[b, nfull * 128 :],
                    out_sb[:rem, nfull * _D : nfull * _D + _D],
                )

    nc.compile()
    return nc


def _host_prep(text, durs, embed, Tt):
    """Per-core input maps + contribution spans (chunk-index ranges)."""
    text = np.asarray(text).astype(np.float32)          # [32, 256]
    durs_f = np.asarray(durs).astype(np.float32)        # [32, 256]
    embed = np.asarray(embed, dtype=np.float32)         # [100, 384]

    T2 = Tt + (Tt % 2)
    NT = (Tt + 127) // 128
    NTP = NT * 128
    grid = _grid(T2)
    NCH = len(grid) - 1

    csum = np.cumsum(durs_f, axis=-1, dtype=np.float32)
    c = csum - durs_f / 2.0                             # centers
    sig = durs_f / 2.0 + _EPS
    sq2 = np.float32(np.sqrt(2.0))
    s_coef = (1.0 / (sig * sq2)).astype(np.float32)
    b_coef = (-c / (sig * sq2)).astype(np.float32)
    amp = (1.0 / (2.0 * sq2 * sig)).astype(np.float32)
    td = np.asarray(durs).astype(np.int64).sum(axis=-1)  # [32]

    # contribution spans per (batch, char-half): t where some |z| < _MARGIN
    lo_t = np.clip(c - _MARGIN * sig, 0, T2).reshape(_B, 2, 128).min(axis=2)
    hi_t = np.clip(c + _MARGIN * sig + 1, 0, T2).reshape(_B, 2, 128).max(axis=2)
    # union across the 8 cores (program is SPMD-shared), snap to grid chunks
    lo_s = lo_t.reshape(_NC, _BPC, 2).min(axis=0)        # [BPC, 2]
    hi_s = hi_t.reshape(_NC, _BPC, 2).max(axis=0)
    garr = np.asarray(grid)
    spans = []
    for b in range(_BPC):
        row = []
        for q in range(2):
            ci_lo = int(np.searchsorted(garr, lo_s[b, q], side="right") - 1)
            ci_hi = int(np.searchsorted(garr, hi_s[b, q], side="left"))
            ci_lo = max(0, min(ci_lo, NCH - 1))
            ci_hi = max(ci_lo + 1, min(ci_hi, NCH))
            row.append((ci_lo, ci_hi))
        spans.append(tuple(row))
    spans = tuple(spans)

    # coef layout: [128 partitions, (b, q, c)] with c = (text, s, b, amp)
    stack = np.stack([text, s_coef, b_coef, amp], axis=-1)   # [32, 256, 4]
    stack = stack.reshape(_B, 2, 128, 4)                      # [32, q, p, c]

    vrow = np.broadcast_to(
        np.arange(_VP, dtype=np.float32), (128, _VP)
    ).copy()

    # embp rows: 0..99 embed, 100 embed[0], 101..127 zero
    # cols: 0..383 embedding, 384 ones (row-sum), 385 zero pad
    embp = np.zeros((_VP, _D + 2), np.float32)
    embp[:_V, :_D] = embed
    embp[:_V, _D] = 1.0
    embp[_V, :_D] = embed[0]
    embp[_V, _D] = 1.0

    t_idx = np.arange(NTP)[None, :]
    beta_all = np.where(t_idx >= td[:, None], _BIG, np.float32(0.0)).astype(
        np.float32
    )                                                         # [32, NTP]

    in_maps = []
    for core in range(_NC):
        bs = slice(core * _BPC, (core + 1) * _BPC)
        coef_core = (
            stack[bs].transpose(2, 0, 1, 3).reshape(128, _BPC * 2 * 4).copy()
        )
        in_maps.append(
            {
                "coef": coef_core,
                "vrow": vrow,
                "embp": embp,
                "beta": beta_all[bs].copy(),
            }
        )
    return in_maps, spans


def kernel(text, durs, embed, total_time):
    global LAST_RESULT
    from concourse.bass_utils import run_bass_kernel_spmd

    Tt = int(total_time)
    in_maps, spans = _host_prep(text, durs, embed, Tt)
    nc = _build_program(Tt, spans)

    trace = bool(int(os.environ.get("GK_TRACE", "0")))
    res = run_bass_kernel_spmd(
        nc, in_maps, list(range(_NC)), trace=trace
    )
    LAST_RESULT = res
    out = np.concatenate([r["out"] for r in res.results], axis=0)
    return out.astype(np.float32)


if __name__ == "__main__":
    rng = np.random.default_rng(0)
    text = rng.integers(1, _V, size=(_B, _T), dtype=np.int64)
    durs = rng.integers(1, 9, size=(_B, _T), dtype=np.int32)
    embed = rng.normal(size=(_V, _D)).astype(np.float32)
    Tt = int(durs.sum(axis=-1).max())
    o = kernel(text, durs, embed, Tt)
    print("out", o.shape, o.dtype)


# revision 12
# speedup vs baseline: 1.2826x; 1.2624x over previous
"""Gaussian upsampling embedding kernel for Trainium2 (8 NeuronCores).

Data-parallel over the batch dim: 32 batches -> 4 per core.

Math (per batch b):
  c_i   = cumsum(durs)_i - durs_i/2          (gaussian centers)
  sig_i = durs_i/2 + 1e-6
  w[t,i] = 1/(sig_i*sqrt(2pi)) * exp(-((t+0.5-c_i)/sig_i)^2/2)
  out[t,:] = sum_i w[t,i]*embed[text_i] / sum_i w[t,i]          (t < total_dur)
  out[t,:] = embed[0]                                           (t >= total_dur)

Device pipeline per batch (engines overlap under Tile):
  ACT : g[i,t] = Derivative_Erf(s_i*tval[t] + b_i)  (= 2/sqrt(pi)*exp(-z^2/2)),
        restricted to the t-span where some char of the half has |z| < 15
        (outside, w underflows to exactly 0 in f32 = reference behavior)
  PE  : O[t,:] = sum over char halves q of g_q[:,tchunk]^T @ Eg_q
        Eg_q[i,:] = amp_i * embed[text_i] with an extra amp_i column
        (-> O[:,384] = row-sum S); halves whose span misses the chunk skip.
        On the trailing chunks (which contain time-padding rows for some
        batch) one extra K=1 matmul adds beta[t] * emb0row,
        beta = BIG*(t>=total_dur): pad rows become BIG*emb0/BIG = embed[0].
  DVE : recip[t] = 1/S (two 128-row chunks per op via strided PSUM AP)
  DVE/ACT : out = O[:,:384]*recip  (psum->sbuf copy fused with normalize;
        whole chunks alternate between the engines to amortize op overhead)

float32r matmul ISA restrictions (s3d3_mm_fp32r_restrictions): stationary
operand must span all 128 PE columns (col_grp=0xf), moving/dst element counts
even -> every matmul has M=128 (t-chunks padded), N=386.
"""

import os
import numpy as np
from contextlib import ExitStack

_B, _T, _V, _D = 32, 256, 100, 384
_NC = 8
_BPC = _B // _NC    # batches per core
_BIG = np.float32(1.0e6)
_EPS = np.float32(1e-6)
_MARGIN = 15.0      # |z| beyond which exp(-z^2/2) is exactly 0 in f32

# normalize chunks handled by ACT (rest on DVE); per-batch chunk index
_ACT_NORM = frozenset((1, 3, 5, 8))

# Set by kernel() after each run (for the local test harness).
LAST_RESULT = None


def _build_program(Tt, spans, pad_c0):
    """spans[b][q] = (c_lo, c_hi) 128-chunk index range char half q of
    batch-slot b contributes to (union across cores). pad_c0 = first chunk
    containing time-pad rows for any batch."""
    import concourse.bass as bass
    import concourse.tile as tile
    from concourse import bacc, mybir

    f32 = mybir.dt.float32
    f32r = mybir.dt.float32r
    AF = mybir.ActivationFunctionType

    NT = (Tt + 127) // 128          # output t-chunks of 128 rows
    NTP = NT * 128
    L = (NT - pad_c0) * 128         # beta cols per batch
    nfull = Tt // 128

    nc = bacc.Bacc(
        "TRN2",
        target_bir_lowering=False,
        debug=False,
        num_devices=_NC,
    )

    coef = nc.dram_tensor("coef", [128, _BPC * 2 * 2], f32, kind="ExternalInput").ap()
    egp = nc.dram_tensor(
        "egp", [_BPC, 2, 128, _D + 2], f32r, kind="ExternalInput"
    ).ap()
    emb0p = nc.dram_tensor("emb0p", [1, _D + 2], f32r, kind="ExternalInput").ap()
    beta = nc.dram_tensor("beta", [1, _BPC * L], f32r, kind="ExternalInput").ap()
    out = nc.dram_tensor("out", [_BPC, Tt, _D], f32, kind="ExternalOutput").ap()

    with tile.TileContext(nc) as tc, ExitStack() as ctx:
        const = ctx.enter_context(tc.tile_pool(name="const", bufs=1))
        wpool = ctx.enter_context(tc.tile_pool(name="wT", bufs=8))
        opool = ctx.enter_context(tc.tile_pool(name="osb", bufs=3))
        rpool = ctx.enter_context(tc.tile_pool(name="recip", bufs=10))
        pso = ctx.enter_context(tc.tile_pool(name="pso", bufs=4, space="PSUM"))

        # constants
        coef_sb = const.tile([128, _BPC * 2 * 2], f32)
        nc.sync.dma_start(coef_sb[:], coef[:])
        eg_sb = const.tile([128, _BPC * 2 * (_D + 2)], f32r)
        nc.sync.dma_start(
            eg_sb[:].rearrange("p (b q d) -> p b q d", q=2, d=_D + 2),
            egp.rearrange("b q p d -> p b q d"),
        )
        emb0_sb = const.tile([1, _D + 2], f32r)
        nc.sync.dma_start(emb0_sb[:], emb0p[:])
        beta_sb = const.tile([1, _BPC * L], f32r)
        nc.sync.dma_start(beta_sb[:], beta[:])
        # tval = arange(NTP)+0.5 replicated on all partitions, built on device
        tvi = const.tile([128, NTP], mybir.dt.int32)
        nc.gpsimd.iota(tvi[:], [[1, NTP]], channel_multiplier=0)
        tval_sb = const.tile([128, NTP], f32)
        nc.vector.tensor_scalar_add(tval_sb[:], tvi[:], 0.5)

        def cf(b, q, c):
            j = (b * 2 + q) * 2 + c
            return coef_sb[:, j : j + 1]

        def eg(b, q):
            j = (b * 2 + q) * (_D + 2)
            return eg_sb[:, j : j + _D + 2]

        for b in range(_BPC):
            # Gaussian eval restricted to contributing span
            wT = []
            for q in range(2):
                lo, hi = spans[b][q][0] * 128, spans[b][q][1] * 128
                w = wpool.tile([128, NTP], f32r, tag="wT")
                nc.scalar.activation(
                    w[:, lo:hi],
                    tval_sb[:, lo:hi],
                    AF.Derivative_Erf,
                    scale=cf(b, q, 0),
                    bias=cf(b, q, 1),
                )
                wT.append(w)

            # out chunks: O[t, 0:384] unnormalized embedding, O[t, 384] = S
            out_sb = opool.tile([128, NT * _D], f32)
            for g in range((NT + 1) // 2):
                ilist = [i for i in (2 * g, 2 * g + 1) if i < NT]
                po = pso.tile([128, 1024], f32, tag="pso")
                for j, i in enumerate(ilist):
                    dst = po[:, j * 512 : j * 512 + _D + 2]
                    qs = [
                        q
                        for q in range(2)
                        if spans[b][q][0] <= i < spans[b][q][1]
                    ]
                    parts = len(qs) + (1 if i >= pad_c0 else 0)
                    assert parts, f"t-chunk {i} has no contribution"
                    k = 0
                    for q in qs:
                        nc.tensor.matmul(
                            dst,
                            wT[q][:, i * 128 : (i + 1) * 128],
                            eg(b, q),
                            start=(k == 0),
                            stop=(k == parts - 1),
                        )
                        k += 1
                    if i >= pad_c0:
                        o0 = b * L + (i - pad_c0) * 128
                        nc.tensor.matmul(
                            dst,
                            beta_sb[0:1, o0 : o0 + 128],
                            emb0_sb[:],
                            start=(k == 0),
                            stop=True,
                        )
                rc = rpool.tile([128, 2], f32, tag="recip")
                ng = len(ilist)
                nc.vector.reciprocal(
                    rc[:, :ng], po[:, _D : _D + 512 * (ng - 1) + 1 : 512]
                )
                for j, i in enumerate(ilist):
                    dst = out_sb[:, i * _D : (i + 1) * _D]
                    src = po[:, j * 512 : j * 512 + _D]
                    if i % 10 in _ACT_NORM:
                        nc.scalar.activation(
                            dst, src, AF.Copy, scale=rc[:, j : j + 1]
                        )
                    else:
                        nc.vector.tensor_scalar_mul(dst, src, rc[:, j : j + 1])

            # store (two halves so the first can flush early)
            h = nfull // 2
            if h:
                nc.sync.dma_start(
                    out[b, : h * 128].rearrange("(i p) d -> p i d", p=128),
                    out_sb[:, : h * _D].rearrange("p (i d) -> p i d", d=_D),
                )
            if nfull > h:
                nc.sync.dma_start(
                    out[b, h * 128 : nfull * 128].rearrange(
                        "(i p) d -> p i d", p=128
                    ),
                    out_sb[:, h * _D : nfull * _D].rearrange(
                        "p (i d) -> p i d", d=_D
                    ),
                )
            if Tt > nfull * 128:
                rem = Tt - nfull * 128
                nc.sync.dma_start(
                    out[b, nfull * 128 :],
                    out_sb[:rem, nfull * _D : nfull * _D + _D],
                )

    nc.compile()
    return nc


def _host_prep(text, durs, embed, Tt):
    """Per-core input maps + per-slot contribution spans on the 128-grid."""
    text_i = np.asarray(text).astype(np.int64)          # [32, 256]
    durs_f = np.asarray(durs).astype(np.float32)        # [32, 256]
    embed = np.asarray(embed, dtype=np.float32)         # [100, 384]

    NT = (Tt + 127) // 128
    NTP = NT * 128

    csum = np.cumsum(durs_f, axis=-1, dtype=np.float32)
    c = csum - durs_f / 2.0                             # centers
    sig = durs_f / 2.0 + _EPS
    sq2 = np.float32(np.sqrt(2.0))
    s_coef = (1.0 / (sig * sq2)).astype(np.float32)
    b_coef = (-c / (sig * sq2)).astype(np.float32)
    amp = (1.0 / (2.0 * sq2 * sig)).astype(np.float32)
    td = np.asarray(durs).astype(np.int64).sum(axis=-1)  # [32]
    pad_c0 = int(td.min()) // 128

    # contribution spans per (batch, char-half) on the 128-chunk grid,
    # unioned across the 8 cores (SPMD-shared program)
    lo_t = np.clip(c - _MARGIN * sig, 0, NTP).reshape(_B, 2, 128).min(axis=2)
    hi_t = np.clip(c + _MARGIN * sig + 1, 0, NTP).reshape(_B, 2, 128).max(axis=2)
    lo_s = lo_t.reshape(_NC, _BPC, 2).min(axis=0)        # [BPC, 2]
    hi_s = hi_t.reshape(_NC, _BPC, 2).max(axis=0)
    spans = []
    for b in range(_BPC):
        row = []
        for q in range(2):
            c_lo = max(0, min(int(lo_s[b, q]) // 128, NT - 1))
            c_hi = max(c_lo + 1, min(-(-int(hi_s[b, q]) // 128), NT))
            row.append((c_lo, c_hi))
        spans.append(tuple(row))
    spans = tuple(spans)

    # coef layout: [128 partitions, (b, q, c)] with c = (s, b)
    stack = np.stack([s_coef, b_coef], axis=-1)          # [32, 256, 2]
    stack = stack.reshape(_B, 2, 128, 2)                 # [32, q, p, c]

    # gathered, amplitude-folded embeddings + amp column (row-sum) + zero pad
    egp = np.zeros((_B, 2, 128, _D + 2), np.float32)
    gat = embed[text_i]                                  # [32, 256, 384]
    egp[:, :, :, :_D] = (gat * amp[:, :, None]).reshape(_B, 2, 128, _D)
    egp[:, :, :, _D] = amp.reshape(_B, 2, 128)

    emb0p = np.zeros((1, _D + 2), np.float32)
    emb0p[0, :_D] = embed[0]
    emb0p[0, _D] = 1.0

    L = (NT - pad_c0) * 128
    t_idx = pad_c0 * 128 + np.arange(L)[None, :]
    beta_all = np.where(t_idx >= td[:, None], _BIG, np.float32(0.0)).astype(
        np.float32
    )                                                    # [32, L]

    in_maps = []
    for core in range(_NC):
        bs = slice(core * _BPC, (core + 1) * _BPC)
        coef_core = (
            stack[bs].transpose(2, 0, 1, 3).reshape(128, _BPC * 2 * 2).copy()
        )
        in_maps.append(
            {
                "coef": coef_core,
                "egp": egp[bs].copy(),
                "emb0p": emb0p,
                "beta": beta_all[bs].reshape(1, -1).copy(),
            }
        )
    return in_maps, spans, pad_c0


def kernel(text, durs, embed, total_time):
    global LAST_RESULT
    from concourse.bass_utils import run_bass_kernel_spmd

    Tt = int(total_time)
    in_maps, spans, pad_c0 = _host_prep(text, durs, embed, Tt)
    nc = _build_program(Tt, spans, pad_c0)

    trace = bool(int(os.environ.get("GK_TRACE", "0")))
    res = run_bass_kernel_spmd(
        nc, in_maps, list(range(_NC)), trace=trace
    )
    LAST_RESULT = res
    out = np.concatenate([r["out"] for r in res.results], axis=0)
    return out.astype(np.float32)


if __name__ == "__main__":
    rng = np.random.default_rng(0)
    text = rng.integers(1, _V, size=(_B, _T), dtype=np.int64)
    durs = rng.integers(1, 9, size=(_B, _T), dtype=np.int32)
    embed = rng.normal(size=(_V, _D)).astype(np.float32)
    Tt = int(durs.sum(axis=-1).max())
    o = kernel(text, durs, embed, Tt)
    print("out", o.shape, o.dtype)


# revision 13
# speedup vs baseline: 1.4320x; 1.1165x over previous
"""Gaussian upsampling embedding kernel for Trainium2 (8 NeuronCores).

Data-parallel over the batch dim: 32 batches -> 4 per core.

Math (per batch b):
  c_i   = cumsum(durs)_i - durs_i/2          (gaussian centers)
  sig_i = durs_i/2 + 1e-6
  w[t,i] = 1/(sig_i*sqrt(2pi)) * exp(-((t+0.5-c_i)/sig_i)^2/2)
  out[t,:] = sum_i w[t,i]*embed[text_i] / sum_i w[t,i]          (t < total_dur)
  out[t,:] = embed[0]                                           (t >= total_dur)

Device pipeline per batch (engines overlap under Tile):
  ACT : g[i,t] = Derivative_Erf(s_i*tval[t] + b_i)  (= 2/sqrt(pi)*exp(-z^2/2)),
        restricted to the t-span where some char of the half has |z| < 15
        (outside, w underflows to exactly 0 in f32 = reference behavior)
  PE  : O[t,:] = sum over char halves q of g_q[:,tchunk]^T @ Eg_q
        Eg_q[i,:] = amp_i * embed[text_i] with an extra amp_i column
        (-> O[:,384] = row-sum S); halves whose span misses the chunk skip.
        On the trailing chunks (which contain time-padding rows for some
        batch) one extra K=1 matmul adds beta[t] * emb0row,
        beta = BIG*(t>=total_dur): pad rows become BIG*emb0/BIG = embed[0].
  DVE : recip[t] = 1/S (two 128-row chunks per op via strided PSUM AP)
  DVE/ACT : out = O[:,:384]*recip  (psum->sbuf copy fused with normalize;
        whole chunks alternate between the engines to amortize op overhead)

float32r matmul ISA restrictions (s3d3_mm_fp32r_restrictions): stationary
operand must span all 128 PE columns (col_grp=0xf), moving/dst element counts
even -> every matmul has M=128 (t-chunks padded), N=386.
"""

import os
import numpy as np
from contextlib import ExitStack

_B, _T, _V, _D = 32, 256, 100, 384
_NC = 8
_BPC = _B // _NC    # batches per core
_BIG = np.float32(1.0e6)
_EPS = np.float32(1e-6)
_MARGIN = 15.0      # |z| beyond which exp(-z^2/2) is exactly 0 in f32

# normalize chunks handled by ACT (rest on DVE); per-batch chunk index
_ACT_NORM = frozenset((1, 3, 5, 8))

# Set by kernel() after each run (for the local test harness).
LAST_RESULT = None


def _build_program(Tt, spans, pad_c0):
    """spans[b][q] = (c_lo, c_hi) 128-chunk index range char half q of
    batch-slot b contributes to (union across cores). pad_c0 = first chunk
    containing time-pad rows for any batch."""
    import concourse.bass as bass
    import concourse.tile as tile
    from concourse import bacc, mybir

    f32 = mybir.dt.float32
    f32r = mybir.dt.float32r
    AF = mybir.ActivationFunctionType

    NT = (Tt + 127) // 128          # output t-chunks of 128 rows
    NTP = NT * 128
    L = (NT - pad_c0) * 128         # beta cols per batch
    nfull = Tt // 128

    nc = bacc.Bacc(
        "TRN2",
        target_bir_lowering=False,
        debug=False,
        num_devices=_NC,
    )

    coef = nc.dram_tensor("coef", [128, _BPC * 2 * 2], f32, kind="ExternalInput").ap()
    egp = nc.dram_tensor(
        "egp", [_BPC, 2, 128, _D + 2], f32r, kind="ExternalInput"
    ).ap()
    emb0p = nc.dram_tensor("emb0p", [1, _D + 2], f32r, kind="ExternalInput").ap()
    beta = nc.dram_tensor("beta", [1, _BPC * L], f32r, kind="ExternalInput").ap()
    out = nc.dram_tensor("out", [_BPC, Tt, _D], f32, kind="ExternalOutput").ap()

    with tile.TileContext(nc) as tc, ExitStack() as ctx:
        const = ctx.enter_context(tc.tile_pool(name="const", bufs=1))
        wpool = ctx.enter_context(tc.tile_pool(name="wT", bufs=8))
        opool = ctx.enter_context(tc.tile_pool(name="osb", bufs=3))
        rpool = ctx.enter_context(tc.tile_pool(name="recip", bufs=10))
        pso = ctx.enter_context(tc.tile_pool(name="pso", bufs=4, space="PSUM"))

        # constants
        coef_sb = const.tile([128, _BPC * 2 * 2], f32)
        nc.sync.dma_start(coef_sb[:], coef[:])
        eg_sb = const.tile([128, _BPC * 2 * (_D + 2)], f32r)
        for bb in range(_BPC):
            w0 = bb * 2 * (_D + 2)
            nc.sync.dma_start(
                eg_sb[:, w0 : w0 + 2 * (_D + 2)].rearrange(
                    "p (q d) -> p q d", q=2
                ),
                egp[bb].rearrange("q p d -> p q d"),
            )
        emb0_sb = const.tile([1, _D + 2], f32r)
        nc.sync.dma_start(emb0_sb[:], emb0p[:])
        beta_sb = const.tile([1, _BPC * L], f32r)
        nc.sync.dma_start(beta_sb[:], beta[:])
        # tval = arange(NTP) replicated on all partitions (f32 iota is exact
        # below 2^24); the +0.5 frame offset is folded into b_coef on host
        tval_sb = const.tile([128, NTP], f32)
        nc.gpsimd.iota(
            tval_sb[:], [[1, NTP]], channel_multiplier=0,
            allow_small_or_imprecise_dtypes=True,
        )

        def cf(b, q, c):
            j = (b * 2 + q) * 2 + c
            return coef_sb[:, j : j + 1]

        def eg(b, q):
            j = (b * 2 + q) * (_D + 2)
            return eg_sb[:, j : j + _D + 2]

        for b in range(_BPC):
            # Gaussian eval restricted to contributing span
            wT = []
            for q in range(2):
                lo, hi = spans[b][q][0] * 128, spans[b][q][1] * 128
                w = wpool.tile([128, NTP], f32r, tag="wT")
                nc.scalar.activation(
                    w[:, lo:hi],
                    tval_sb[:, lo:hi],
                    AF.Derivative_Erf,
                    scale=cf(b, q, 0),
                    bias=cf(b, q, 1),
                )
                wT.append(w)

            # out chunks: O[t, 0:384] unnormalized embedding, O[t, 384] = S
            out_sb = opool.tile([128, NT * _D], f32)
            for g in range((NT + 1) // 2):
                ilist = [i for i in (2 * g, 2 * g + 1) if i < NT]
                po = pso.tile([128, 1024], f32, tag="pso")
                for j, i in enumerate(ilist):
                    dst = po[:, j * 512 : j * 512 + _D + 2]
                    qs = [
                        q
                        for q in range(2)
                        if spans[b][q][0] <= i < spans[b][q][1]
                    ]
                    parts = len(qs) + (1 if i >= pad_c0 else 0)
                    assert parts, f"t-chunk {i} has no contribution"
                    k = 0
                    for q in qs:
                        nc.tensor.matmul(
                            dst,
                            wT[q][:, i * 128 : (i + 1) * 128],
                            eg(b, q),
                            start=(k == 0),
                            stop=(k == parts - 1),
                        )
                        k += 1
                    if i >= pad_c0:
                        o0 = b * L + (i - pad_c0) * 128
                        nc.tensor.matmul(
                            dst,
                            beta_sb[0:1, o0 : o0 + 128],
                            emb0_sb[:],
                            start=(k == 0),
                            stop=True,
                        )
                rc = rpool.tile([128, 2], f32, tag="recip")
                ng = len(ilist)
                nc.vector.reciprocal(
                    rc[:, :ng], po[:, _D : _D + 512 * (ng - 1) + 1 : 512]
                )
                for j, i in enumerate(ilist):
                    dst = out_sb[:, i * _D : (i + 1) * _D]
                    src = po[:, j * 512 : j * 512 + _D]
                    if i % 10 in _ACT_NORM:
                        nc.scalar.activation(
                            dst, src, AF.Copy, scale=rc[:, j : j + 1]
                        )
                    else:
                        nc.vector.tensor_scalar_mul(dst, src, rc[:, j : j + 1])

            # store (two halves so the first can flush early)
            h = nfull // 2
            if h:
                nc.sync.dma_start(
                    out[b, : h * 128].rearrange("(i p) d -> p i d", p=128),
                    out_sb[:, : h * _D].rearrange("p (i d) -> p i d", d=_D),
                )
            if nfull > h:
                nc.sync.dma_start(
                    out[b, h * 128 : nfull * 128].rearrange(
                        "(i p) d -> p i d", p=128
                    ),
                    out_sb[:, h * _D : nfull * _D].rearrange(
                        "p (i d) -> p i d", d=_D
                    ),
                )
            if Tt > nfull * 128:
                rem = Tt - nfull * 128
                nc.sync.dma_start(
                    out[b, nfull * 128 :],
                    out_sb[:rem, nfull * _D : nfull * _D + _D],
                )

    nc.compile()
    return nc


def _host_prep(text, durs, embed, Tt):
    """Per-core input maps + per-slot contribution spans on the 128-grid."""
    text_i = np.asarray(text).astype(np.int64)          # [32, 256]
    durs_f = np.asarray(durs).astype(np.float32)        # [32, 256]
    embed = np.asarray(embed, dtype=np.float32)         # [100, 384]

    NT = (Tt + 127) // 128
    NTP = NT * 128

    csum = np.cumsum(durs_f, axis=-1, dtype=np.float32)
    c = csum - durs_f / 2.0                             # centers
    sig = durs_f / 2.0 + _EPS
    sq2 = np.float32(np.sqrt(2.0))
    s_coef = (1.0 / (sig * sq2)).astype(np.float32)
    b_coef = ((0.5 - c) / (sig * sq2)).astype(np.float32)
    amp = (1.0 / (2.0 * sq2 * sig)).astype(np.float32)
    td = np.asarray(durs).astype(np.int64).sum(axis=-1)  # [32]
    pad_c0 = int(td.min()) // 128

    # contribution spans per (batch, char-half) on the 128-chunk grid,
    # unioned across the 8 cores (SPMD-shared program)
    lo_t = np.clip(c - _MARGIN * sig, 0, NTP).reshape(_B, 2, 128).min(axis=2)
    hi_t = np.clip(c + _MARGIN * sig + 1, 0, NTP).reshape(_B, 2, 128).max(axis=2)
    lo_s = lo_t.reshape(_NC, _BPC, 2).min(axis=0)        # [BPC, 2]
    hi_s = hi_t.reshape(_NC, _BPC, 2).max(axis=0)
    spans = []
    for b in range(_BPC):
        row = []
        for q in range(2):
            c_lo = max(0, min(int(lo_s[b, q]) // 128, NT - 1))
            c_hi = max(c_lo + 1, min(-(-int(hi_s[b, q]) // 128), NT))
            row.append((c_lo, c_hi))
        spans.append(tuple(row))
    spans = tuple(spans)

    # coef layout: [128 partitions, (b, q, c)] with c = (s, b)
    stack = np.stack([s_coef, b_coef], axis=-1)          # [32, 256, 2]
    stack = stack.reshape(_B, 2, 128, 2)                 # [32, q, p, c]

    # gathered, amplitude-folded embeddings + amp column (row-sum) + zero pad
    egp = np.zeros((_B, 2, 128, _D + 2), np.float32)
    gat = embed[text_i]                                  # [32, 256, 384]
    egp[:, :, :, :_D] = (gat * amp[:, :, None]).reshape(_B, 2, 128, _D)
    egp[:, :, :, _D] = amp.reshape(_B, 2, 128)

    emb0p = np.zeros((1, _D + 2), np.float32)
    emb0p[0, :_D] = embed[0]
    emb0p[0, _D] = 1.0

    L = (NT - pad_c0) * 128
    t_idx = pad_c0 * 128 + np.arange(L)[None, :]
    beta_all = np.where(t_idx >= td[:, None], _BIG, np.float32(0.0)).astype(
        np.float32
    )                                                    # [32, L]

    in_maps = []
    for core in range(_NC):
        bs = slice(core * _BPC, (core + 1) * _BPC)
        coef_core = (
            stack[bs].transpose(2, 0, 1, 3).reshape(128, _BPC * 2 * 2).copy()
        )
        in_maps.append(
            {
                "coef": coef_core,
                "egp": egp[bs].copy(),
                "emb0p": emb0p,
                "beta": beta_all[bs].reshape(1, -1).copy(),
            }
        )
    return in_maps, spans, pad_c0


def kernel(text, durs, embed, total_time):
    global LAST_RESULT
    from concourse.bass_utils import run_bass_kernel_spmd

    Tt = int(total_time)
    in_maps, spans, pad_c0 = _host_prep(text, durs, embed, Tt)
    nc = _build_program(Tt, spans, pad_c0)

    trace = bool(int(os.environ.get("GK_TRACE", "0")))
    res = run_bass_kernel_spmd(
        nc, in_maps, list(range(_NC)), trace=trace
    )
    LAST_RESULT = res
    out = np.concatenate([r["out"] for r in res.results], axis=0)
    return out.astype(np.float32)


if __name__ == "__main__":
    rng = np.random.default_rng(0)
    text = rng.integers(1, _V, size=(_B, _T), dtype=np.int64)
    durs = rng.integers(1, 9, size=(_B, _T), dtype=np.int32)
    embed = rng.normal(size=(_V, _D)).astype(np.float32)
    Tt = int(durs.sum(axis=-1).max())
    o = kernel(text, durs, embed, Tt)
    print("out", o.shape, o.dtype)
